# revision 11
# baseline (speedup 1.0000x reference)
"""Self-contained Trainium2 Bass kernel for a single attention head.

Problem: B=8, S=2048, E=1024, D=64 (fp32 in/out).
  q = query @ Wq.T + bq ; k, v likewise
  out = softmax(mask(q @ k.T / sqrt(D))) @ v
  mask = query_mask[:, :, None] * key_mask[:, None, :]; query_mask is all-ones
  per the problem spec (fill="ones").

Sharding: pure data-parallel, one batch element per NeuronCore (8 cores).

Design (v4):
  - fp16 compute, fp32 PSUM accumulation. Host compacts masked key
    columns away (S_k 2048 -> ~1100, padded to 64/128); pad columns get
    exp bias -30000 -> 0.
  - q/k projections use a column-duplicated stationary [W|W] (M=128,
    same cycle cost as M=64) so qT/kT land in BOTH partition halves.
    Scores then run as row-tiled CONCURRENT pairs: chunk c0 in PE rows
    0-63 (kT lo), chunk c1 in rows 64-127 (kT hi); tile_position is
    auto-derived from base partitions -> ~2x score matmul rate.
  - (q2,q3) projection is column-PAIRED: two concurrent M=64 matmuls in
    PE col groups per e-pass; h1 chunk c0 reads lo rows, c1 hi rows.
  - ~22 warm-up matmuls bridge the DMA ramp so the PE HAM clock stays
    at 2.4 GHz into the first real projection.
  - Every engine queue is FIFO: a blocked instruction stalls everything
    behind it on that engine. Scores are exp-paced (~1.06us per ssT
    ring slot), so filler work (later projections, transposes, AV) is
    emitted in <=~0.9us chunks BEFORE each blocking score, placed so a
    late DMA can't stall the score chain.
  - PSUM: 2-slot fp32 ssT ring (4 banks) + 2-slot proj/transpose pool
    (2 banks) + num0 accumulator [65,1024] (2 banks) = 8. num0 is
    shared h0 -> h1: chunked copies release each bank to AV-h1 right
    after AV-h0's last matmul on it.
  - exp-chain critical-path bias adds are split DVE+ACT (ACT is idle
    before the first exp). q half0 arrives as 512/256/256-col pieces so
    only a 256-col projection sits between the last q byte and the
    first score.
  - No on-chip normalize/transpose finale: raw [65, S] numerator rows
    go PSUM -> SBUF fp16 -> DRAM; the host does (num[:64]/num[64]).T.
"""

from contextlib import ExitStack

import numpy as np

import concourse.bass as bass
import concourse.mybir as mybir
import concourse.tile as tile
from concourse import bacc
from concourse.bass_utils import run_bass_kernel_spmd
from concourse.masks import make_identity

FP16 = mybir.dt.float16
F32 = mybir.dt.float32

N_CORES = 8
B, S, E, D = 8, 2048, 1024, 64
P = 128
NE = E // P            # 8 contraction tiles
NH = 2                 # query halves
HI = S // NH           # 1024 query positions per half
NC = 512               # matmul free-dim chunk (one PSUM bank of f32)
SCALE = 1.0 / np.sqrt(np.float32(D))
MASK_NEG = -30000.0
N_WARM = 22

QP0 = [(0, 512), (512, 256), (768, 256)]   # q half-0 staging pieces


def _chunks(total, step, base=0):
    out = []
    o = 0
    while o < total:
        out.append((base + o, min(step, total - o)))
        o += step
    return out


def _build(tc: tile.TileContext, ins: dict, out_d: bass.AP, ctx, sk2: int,
           nkr: int):
    nc = tc.nc
    nj = sk2 // P
    kp = _chunks(nkr, NC)
    vp = _chunks(nkr, NC)

    consts = ctx.enter_context(tc.tile_pool(name="consts", bufs=1))
    stage = ctx.enter_context(tc.tile_pool(name="stage", bufs=1))
    proj = ctx.enter_context(tc.tile_pool(name="proj", bufs=1))
    xpool = ctx.enter_context(tc.tile_pool(name="xpool", bufs=max(nj, 2)))
    ppool = ctx.enter_context(tc.tile_pool(name="ppool", bufs=max(2 * nj, 2)))
    fin = ctx.enter_context(tc.tile_pool(name="fin", bufs=1))
    ps_mm = ctx.enter_context(tc.tile_pool(name="ps_mm", bufs=2, space="PSUM"))
    ps_sm = ctx.enter_context(tc.tile_pool(name="ps_sm", bufs=2, space="PSUM"))
    ps_acc = ctx.enter_context(tc.tile_pool(name="ps_acc", bufs=1,
                                            space="PSUM"))

    # --- staged inputs, HWDGE SP ring, consumption-deadline order --------
    wqd = consts.tile([P, NE * P], FP16, tag="wqd")
    wkv = consts.tile([P, NE * P + NE * D], FP16, tag="wkv")
    c32 = consts.tile([P, nj + 3], F32, tag="c32")
    qsh = {i: stage.tile([P, NE * n], FP16, tag=f"q{i}", name=f"qs{i}")
           for i, (o, n) in enumerate(QP0)}
    q23s = stage.tile([P, NE * 2 * NC], FP16, tag="q23s")
    ksh = {i: stage.tile([P, NE * kp[i][1]], FP16, tag=f"k{i}",
                         name=f"ks{i}") for i in range(len(kp))}
    vsh = {i: stage.tile([P, NE * vp[i][1]], FP16, tag=f"v{i}",
                         name=f"vs{i}") for i in range(len(vp))}

    nc.sync.dma_start(out=wqd[:], in_=ins["wqd"][:])
    nc.sync.dma_start(out=wkv[:], in_=ins["wkv"][:])
    nc.sync.dma_start(out=c32[:], in_=ins["c32"][:])
    nc.sync.dma_start(out=qsh[0][:], in_=ins["q0"][:])
    nc.sync.dma_start(out=ksh[0][:], in_=ins["k0"][:])
    nc.sync.dma_start(out=qsh[1][:], in_=ins["q1"][:])
    nc.sync.dma_start(out=qsh[2][:], in_=ins["q2"][:])
    for i in range(1, len(kp)):
        nc.sync.dma_start(out=ksh[i][:], in_=ins[f"k{i}"][:])
    nc.sync.dma_start(out=q23s[:], in_=ins["q23"][:])
    if len(vp) > 2:
        nc.sync.dma_start(out=vsh[2][:], in_=ins["v2"][:])
    nc.sync.dma_start(out=vsh[0][:], in_=ins["v0"][:])
    if len(vp) > 1:
        nc.sync.dma_start(out=vsh[1][:], in_=ins["v1"][:])

    wkd = wkv[:, 0:NE * P]
    wv = wkv[:, NE * P:NE * P + NE * D]
    mb = c32[:, 0:nj]
    bq = c32[:, nj:nj + 1]          # duplicated rows 0-63 / 64-127
    bk = c32[:, nj + 1:nj + 2]
    bv = c32[0:D, nj + 2:nj + 3]

    # --- engine warm-up / constants --------------------------------------
    ident = consts.tile([P, P], FP16, tag="ident")
    junk = consts.tile([P, NC], FP16, tag="junk")
    warm = consts.tile([P, 16], F32, tag="warm")
    make_identity(nc, ident[:])
    nc.vector.memset(junk[:], 0.0)
    nc.vector.memset(warm[:], 0.0)
    nc.scalar.activation(warm[:], warm[:], mybir.ActivationFunctionType.Exp)

    # persistent projected tensors
    # qT128: cols 0:HI = half0 duplicated in both partition halves;
    #        cols HI:HI+NC = half1 (lo rows = q cols 1024-1536, hi rows =
    #        q cols 1536-2048) from the column-paired (q2,q3) projection.
    qT128 = proj.tile([P, HI + NC], FP16, tag="qT128")
    kT128 = proj.tile([P, sk2], FP16, tag="kT128")
    vT65 = proj.tile([D + 1, sk2], FP16, tag="vT65")
    nc.vector.memset(vT65[D:D + 1, :], 1.0)   # ones row -> softmax denom
    if nkr < sk2:
        nc.vector.memset(kT128[:, nkr:sk2], 0.0)
        nc.vector.memset(vT65[0:D, nkr:sk2], 0.0)

    # num0 allocated first so warm-up matmuls can target its PSUM.
    num0 = ps_acc.tile([D + 1, HI], F32, tag="num", name="num0")
    for w in range(N_WARM):
        nc.tensor.matmul(num0[0:D + 1, 0:NC], ident[:, 0:D + 1], junk[:],
                         start=True, stop=True, skip_group_check=True)

    # ---- projection helpers ---------------------------------------------
    class Proj:
        """One projection piece, emitted in e-pass chunks so passes can
        be packed into exp-paced PE stall gaps.

        mode "dup":  stationary [W|W] M=128; out rows 64-127 copy rows
                     0-63 of W.T @ src.
        mode "pair": two concurrent M=64 col-group matmuls per e-pass
                     (src has per-e layout [piece_a | piece_b]); out lo
                     rows = piece_a, hi rows = piece_b.
        mode "v":    plain M=64 -> vT65 lo rows.
        """

        def __init__(self, mode, dst, dstcol, w, bias_ap, src, n):
            self.__dict__.update(mode=mode, dst=dst, dstcol=dstcol, w=w,
                                 bias_ap=bias_ap, src=src, n=n)
            self.ps = ps_sm.tile([P, NC], F32, tag="ps_sm",
                                 name=f"ps_{mode}_{dstcol}")

        def passes(self, e0, e1):
            n, w, src, ps = self.n, self.w, self.src, self.ps
            for e in range(e0, e1):
                st, sp = e == 0, e == NE - 1
                if self.mode == "dup":
                    nc.tensor.matmul(ps[:, 0:n], w[:, e * P:(e + 1) * P],
                                     src[:, e * n:(e + 1) * n],
                                     start=st, stop=sp)
                elif self.mode == "pair":
                    nc.tensor.matmul(ps[0:D, 0:n], w[:, e * P:e * P + D],
                                     src[:, e * 2 * n:e * 2 * n + n],
                                     start=st, stop=sp)
                    nc.tensor.matmul(ps[D:P, 0:n], w[:, e * P:e * P + D],
                                     src[:, e * 2 * n + n:(e + 1) * 2 * n],
                                     start=st, stop=sp)
                else:
                    nc.tensor.matmul(ps[0:D, 0:n], w[:, e * D:(e + 1) * D],
                                     src[:, e * n:(e + 1) * n],
                                     start=st, stop=sp)

        def adds(self, fast=False):
            n, c0, ps, b = self.n, self.dstcol, self.ps, self.bias_ap
            if self.mode == "dup":
                dst = self.dst[:, c0:c0 + n]
                if fast:
                    h = n // 2
                    nc.vector.tensor_scalar_add(dst[:, 0:h], ps[:, 0:h], b)
                    nc.scalar.add(dst[:, h:n], ps[:, h:n], b)
                else:
                    nc.vector.tensor_scalar_add(dst, ps[:, 0:n], b)
            elif self.mode == "pair":
                nc.vector.tensor_scalar_add(
                    self.dst[0:D, c0:c0 + n], ps[0:D, 0:n], b[0:D])
                nc.vector.tensor_scalar_add(
                    self.dst[D:P, c0:c0 + n], ps[D:P, 0:n], b[D:P])
            else:
                nc.vector.tensor_scalar_add(
                    self.dst[0:D, c0:c0 + n], ps[0:D, 0:n], b)

        def all(self, fast=False):
            self.passes(0, NE)
            self.adds(fast)

    # ---- attention helpers ----------------------------------------------
    pms = {}

    def sc(h, j):
        """Scores for (h, j): two row-tiled concurrent N=512 matmuls into
        one fp32 2-bank PSUM tile, then the exp into SBUF fp16."""
        sst = ps_mm.tile([P, HI], F32, tag="ps_mm", name=f"ssT_{h}_{j}")
        if h == 0:
            qlo = qT128[0:D, 0:NC]
            qhi = qT128[D:P, NC:HI]
        else:
            qlo = qT128[0:D, HI:HI + NC]
            qhi = qT128[D:P, HI:HI + NC]
        nc.tensor.matmul(sst[:, 0:NC], kT128[0:D, j * P:(j + 1) * P], qlo,
                         start=True, stop=True)
        nc.tensor.matmul(sst[:, NC:HI], kT128[D:P, j * P:(j + 1) * P], qhi,
                         start=True, stop=True)
        p = ppool.tile([P, HI], FP16, tag="pm", name=f"pm_{h}_{j}")
        nc.scalar.activation(p[:], sst[:], mybir.ActivationFunctionType.Exp,
                             bias=mb[:, j:j + 1], scale=float(SCALE))
        pms[(h, j)] = p

    xt = [None] * nj

    def x_one(j):
        pst = ps_sm.tile([P, D + 1], FP16, tag="ps_sm", name=f"psx{j}")
        nc.tensor.transpose(pst[:], vT65[:, j * P:(j + 1) * P],
                            ident[0:D + 1, 0:D + 1])
        x = xpool.tile([P, D + 1], FP16, tag="x", name=f"x{j}")
        nc.vector.tensor_copy(x[:], pst[:])
        xt[j] = x

    nsb0 = fin.tile([D + 1, HI], FP16, tag="nsb0")
    nsb1 = fin.tile([D + 1, HI], FP16, tag="nsb1")

    def av(h, j):
        for c in range(HI // NC):
            nc.tensor.matmul(
                num0[:, c * NC:(c + 1) * NC],
                xt[j][:],
                pms[(h, j)][:, c * NC:(c + 1) * NC],
                start=(j == 0), stop=(j == nj - 1),
            )
            if j == nj - 1 and h == 0:
                # release this num0 bank to AV-h1 immediately
                nc.vector.tensor_copy(nsb0[:, c * NC:(c + 1) * NC],
                                      num0[:, c * NC:(c + 1) * NC])

    # ---- emission --------------------------------------------------------
    pq0 = Proj("dup", qT128, QP0[0][0], wqd, bq, qsh[0][:], QP0[0][1])
    pk0 = Proj("dup", kT128, kp[0][0], wkd, bk, ksh[0][:], kp[0][1])
    pq1 = Proj("dup", qT128, QP0[1][0], wqd, bq, qsh[1][:], QP0[1][1])
    pq2 = Proj("dup", qT128, QP0[2][0], wqd, bq, qsh[2][:], QP0[2][1])
    pq0.all(fast=True)
    pk0.all(fast=True)
    pq1.all(fast=True)
    pq2.all(fast=True)
    sc(0, 0)
    sc(0, 1)

    if nj == 9 and len(vp) == 3 and len(kp) == 3:
        pk1 = Proj("dup", kT128, kp[1][0], wkd, bk, ksh[1][:], kp[1][1])
        pk2 = Proj("dup", kT128, kp[2][0], wkd, bk, ksh[2][:], kp[2][1])
        q23 = Proj("pair", qT128, HI, wqd, bq, q23s[:], NC)
        v0 = Proj("v", vT65, vp[0][0], wv, bv, vsh[0][:], vp[0][1])
        v1 = Proj("v", vT65, vp[1][0], wv, bv, vsh[1][:], vp[1][1])
        v2 = Proj("v", vT65, vp[2][0], wv, bv, vsh[2][:], vp[2][1])
        sc(0, 2)
        pk1.passes(0, 4)
        sc(0, 3)
        pk1.passes(4, 8)
        pk1.adds()
        sc(0, 4)
        pk2.all()
        sc(0, 5)
        sc(0, 6)
        q23.passes(0, 3)
        sc(0, 7)
        q23.passes(3, 6)
        sc(0, 8)
        q23.passes(6, 8)
        q23.adds()
        sc(1, 0)
        v2.all()
        x_one(8)
        sc(1, 1)
        v0.passes(0, 4)
        sc(1, 2)
        v0.passes(4, 8)
        v0.adds()
        sc(1, 3)
        x_one(0)
        x_one(1)
        x_one(2)
        x_one(3)
        sc(1, 4)
        av(0, 0)
        av(0, 1)
        sc(1, 5)
        av(0, 2)
        av(0, 3)
        v1.passes(0, 4)
        sc(1, 6)
        v1.passes(4, 8)
        v1.adds()
        sc(1, 7)
        x_one(4)
        x_one(5)
        x_one(6)
        x_one(7)
        sc(1, 8)
        av(0, 4)
        av(0, 5)
        av(0, 6)
        av(0, 7)
        av(0, 8)        # emits the per-chunk nsb0 copies
        nc.sync.dma_start(out=out_d[0:D + 1, :], in_=nsb0[:])
        for j in range(nj):
            av(1, j)
    else:
        for j in range(2, min(4, nj)):
            sc(0, j)
        for i in range(1, len(kp)):
            Proj("dup", kT128, kp[i][0], wkd, bk, ksh[i][:],
                 kp[i][1]).all()
        for j in range(4, nj):
            sc(0, j)
        Proj("pair", qT128, HI, wqd, bq, q23s[:], NC).all()
        for j in range(nj):
            sc(1, j)
        done_x = 0
        for i, (o, n) in enumerate(vp):
            Proj("v", vT65, o, wv, bv, vsh[i][:], n).all()
            hi_j = nj if i == len(vp) - 1 else (o + n) // P
            for j in range(done_x, hi_j):
                x_one(j)
            done_x = hi_j
        for j in range(nj):
            av(0, j)
        nc.sync.dma_start(out=out_d[0:D + 1, :], in_=nsb0[:])
        for j in range(nj):
            av(1, j)

    # tail: chunk c0 copy on ACT (idle after the last exp), c1 on DVE,
    # stores chunked so the first can fly while the second copies.
    nc.scalar.copy(nsb1[:, 0:NC], num0[:, 0:NC])
    nc.sync.dma_start(out=out_d[D + 1:2 * (D + 1), 0:NC],
                      in_=nsb1[:, 0:NC])
    nc.vector.tensor_copy(nsb1[:, NC:HI], num0[:, NC:HI])
    nc.sync.dma_start(out=out_d[D + 1:2 * (D + 1), NC:HI],
                      in_=nsb1[:, NC:HI])


_COMPILED = {}


def _get_compiled(sk2: int, nkr: int):
    key = (sk2, nkr)
    if key not in _COMPILED:
        nj = sk2 // P
        kp = _chunks(nkr, NC)
        vp = _chunks(nkr, NC)
        nc = bacc.Bacc("TRN2", target_bir_lowering=False, debug=False,
                       num_devices=N_CORES)

        def din(name, shape, dt=FP16):
            return nc.dram_tensor(name, shape, dt, kind="ExternalInput").ap()

        ins = {"wqd": din("wqd", [P, NE * P]),
               "wkv": din("wkv", [P, NE * P + NE * D]),
               "c32": din("c32", [P, nj + 3], F32),
               "q23": din("q23", [P, NE * 2 * NC])}
        for i, (o, n) in enumerate(QP0):
            ins[f"q{i}"] = din(f"q{i}", [P, NE * n])
        for pref, pieces in (("k", kp), ("v", vp)):
            for i, (o, n) in enumerate(pieces):
                ins[f"{pref}{i}"] = din(f"{pref}{i}", [P, NE * n])
        out_d = nc.dram_tensor("out", [NH * (D + 1), HI], FP16,
                               kind="ExternalOutput").ap()
        with tile.TileContext(nc) as tc:
            with ExitStack() as ctx:
                _build(tc, ins, out_d, ctx, sk2, nkr)
        nc.compile()
        _COMPILED[key] = nc
    return _COMPILED[key]


def _blob(x16, lo, hi):
    """[S', E] fp16 row-slice -> staging blob [P, NE*(hi-lo)] laid out as
    [partition, e-block, col]."""
    return np.ascontiguousarray(
        x16[lo:hi].reshape(hi - lo, NE, P).transpose(2, 1, 0)
    ).reshape(P, -1)


LAST_RESULTS = None


def kernel(query, key, value, query_mask, key_mask, Wq, bq, Wk, bk, Wv, bv):
    global LAST_RESULTS
    query = np.asarray(query, dtype=np.float32)
    key = np.asarray(key, dtype=np.float32)
    value = np.asarray(value, dtype=np.float32)
    key_mask = np.asarray(key_mask)

    # compact masked keys away (they contribute exactly zero)
    keeps = [np.nonzero(key_mask[c] != 0)[0] for c in range(N_CORES)]
    nk_max = max(len(kps) for kps in keeps)
    sk2 = max(P, int(np.ceil(nk_max / P)) * P)
    sk2 = min(sk2, S)
    nkr = min(sk2, max(P, int(np.ceil(nk_max / 64)) * 64))
    nj = sk2 // P
    kp = _chunks(nkr, NC)
    vp = _chunks(nkr, NC)

    def wblob(w):
        return (np.asarray(w, np.float32).astype(np.float16)
                .reshape(D, NE, P).transpose(2, 1, 0))

    wq3 = wblob(Wq)
    wqd = np.ascontiguousarray(
        np.concatenate([wq3, wq3], axis=2)).reshape(P, NE * P)
    wk3 = wblob(Wk)
    wkd = np.concatenate([wk3, wk3], axis=2).reshape(P, NE * P)
    wv2 = wblob(Wv).reshape(P, NE * D)
    wkv = np.ascontiguousarray(np.concatenate([wkd, wv2], axis=1))

    c32 = np.zeros((P, nj + 3), np.float32)
    for i, b in enumerate((bq, bk)):
        bb = np.asarray(b, np.float32).reshape(D)
        c32[0:D, nj + i] = bb
        c32[D:P, nj + i] = bb
    c32[0:D, nj + 2] = np.asarray(bv, np.float32).reshape(D)

    in_maps = []
    for c in range(N_CORES):
        kps = keeps[c]
        nk = len(kps)
        q16 = query[c].astype(np.float16)
        kc = np.zeros((nkr, E), np.float16)
        vc = np.zeros((nkr, E), np.float16)
        kc[0:nk] = key[c][kps].astype(np.float16)
        vc[0:nk] = value[c][kps].astype(np.float16)
        c32c = c32.copy()
        mbias = np.full(sk2, np.float32(MASK_NEG))
        mbias[0:nk] = 0.0
        c32c[:, 0:nj] = mbias.reshape(nj, P).T
        # q23 blob: [P, NE, 2*NC] with per-e layout [piece2 | piece3]
        b2 = _blob(q16, HI, HI + NC).reshape(P, NE, NC)
        b3 = _blob(q16, HI + NC, S).reshape(P, NE, NC)
        q23 = np.ascontiguousarray(
            np.concatenate([b2, b3], axis=2)).reshape(P, -1)
        im = {"wqd": wqd, "wkv": wkv, "c32": np.ascontiguousarray(c32c),
              "q23": q23}
        for i, (o, n) in enumerate(QP0):
            im[f"q{i}"] = _blob(q16, o, o + n)
        for pref, pieces, arr in (("k", kp, kc), ("v", vp, vc)):
            for i, (o, n) in enumerate(pieces):
                im[f"{pref}{i}"] = _blob(arr, o, o + n)
        in_maps.append(im)

    nc = _get_compiled(sk2, nkr)
    res = run_bass_kernel_spmd(nc, in_maps, core_ids=list(range(N_CORES)))
    LAST_RESULTS = res

    out = np.empty((N_CORES, S, D), np.float32)
    for c in range(N_CORES):
        o = np.asarray(res.results[c]["out"]).astype(np.float32)
        for h in range(NH):
            nh = o[h * (D + 1):(h + 1) * (D + 1)]
            out[c, h * HI:(h + 1) * HI] = (nh[0:D] / nh[D:D + 1]).T
    return out
